# revision 45
# baseline (speedup 1.0000x reference)
"""Segment-mean (MeanToERA5) Trainium2 kernel.

Computes per-cluster means of a [32, 8, 512, 512] fp32 tensor over the
flattened 512x512 spatial axis, for 4096 clusters given by `mapping`
([262144] int), matching jax.ops.segment_sum(flat.T, mapping)/counts.

Strategy (8 NeuronCores, SPMD; the kernel is HBM-bandwidth bound):
  - Host: stable-argsort `mapping`; bin-pack the 4096 clusters into 128
    groups of G=32 with equal row sums (2048 -> zero padding); each core
    owns 512 clusters = 16 groups. Rows are laid out cluster-sorted and
    transposed as [256 batch] vectors, packed partition-major so every
    group is one fully contiguous HBM region fetched by one DMA.
  - Precision: the error gate is 2e-2. Every 4th row of each cluster is
    stored as fp8 e4m3, the rest as bf16 (measured end-to-end rel err
    1.44e-2, exactly reproduced on device since quantization happens on
    host and the device accumulates exactly in fp32 PSUM). This cuts HBM
    traffic to 14.7 MB/core: fp32 would be 33.5 MB, pure bf16 16.8 MB.
  - Device: build 0/1 one-hot weights on DVE from compact column-id
    vectors (fp8 and bf16 variants); per 128-row chunk one matmul:
    stationary = one-hot [128, 32], moving = data chunk [128, 256] viewed
    from the byte-packed tile via bitcast. PSUM accumulates [512 clusters,
    256 batch] c-major in 4 [128, 256] fp32 tiles; scale by per-cluster
    1/count on the psum->sbuf copy (Activation-ring DMAs for side inputs
    and outputs, x fetches alternate between both HWDGE rings), out fp16.
  - Host: assemble [4096, 256], unpermute, transpose (the unshard).
"""

import sys
import time

if "/opt/trn_rl_repo" not in sys.path:
    sys.path.insert(0, "/opt/trn_rl_repo")

import numpy as np
import ml_dtypes
import jax

# Persistent JAX compilation cache: the NEFF compile (~2 min) is reused
# across processes for identical programs.
try:
    if jax.config.jax_compilation_cache_dir is None:
        jax.config.update("jax_compilation_cache_dir", "/tmp/jax_neff_cache")
    jax.config.update("jax_persistent_cache_min_entry_size_bytes", -1)
    jax.config.update("jax_persistent_cache_min_compile_time_secs", 0.1)
except Exception:
    pass

import concourse.bacc as bacc
import concourse.tile as tile
from concourse import mybir
from concourse.bass_utils import run_bass_kernel_spmd

N_CLUSTERS = 4096
N = 512 * 512
B = 256
NCORES = 8
G = 32                      # clusters per group (= one-hot width)
GROUPS_PER_CORE = (N_CLUSTERS // NCORES) // G   # 16
CLUSTERS_PER_CORE = N_CLUSTERS // NCORES        # 512
NQ = CLUSTERS_PER_CORE // 128                   # psum tiles (4)
FG = 1                      # groups per x fetch
XBUFS = 13                  # x tile pool depth
C8 = 5                      # fp8 chunks per group (C8/16 of rows in e4m3;
                            # striped evenly per-cluster so every cluster is
                            # ~31% fp8 -> rel err ~1.47e-2, inside the 2e-2
                            # gate (C8=6 measured 1.79e-2: too thin)

_program_cache = {}
LAST_EXEC_NS = None


def _build_program(cpg: int, loop: int = 1):
    """Build the SPMD bass program for `cpg` 128-row chunks per group.

    loop > 1 repeats the whole pipeline on-device (for benchmarking: one
    dispatch, `loop` executions)."""
    key = (cpg, loop)
    if key in _program_cache:
        return _program_cache[key]

    nchunks = GROUPS_PER_CORE * cpg    # chunks per core
    gpq = 128 // G                     # groups per psum tile (4)

    nc = bacc.Bacc("TRN2", target_bir_lowering=False, debug=False,
                   num_devices=NCORES)
    # x packed per group as raw bytes: per partition, C8 fp8 chunks
    # (C8*B bytes) then (cpg-C8) bf16 chunks ((cpg-C8)*B*2 bytes);
    # host pre-permutes so every fetch is one contiguous region
    c16 = cpg - C8
    r8b = C8 * B                # fp8 region bytes per partition
    gbytes = r8b + c16 * B * 2  # total bytes per partition per group
    nfetch = GROUPS_PER_CORE // FG
    x = nc.dram_tensor("x", [nfetch, 128, gbytes],
                       mybir.dt.uint8, kind="ExternalInput")
    # per-row one-hot column id, packed [128, nchunks]
    cid = nc.dram_tensor("cid", [128, nchunks], mybir.dt.bfloat16,
                         kind="ExternalInput")
    iota = nc.dram_tensor("iota", [128, G], mybir.dt.bfloat16,
                          kind="ExternalInput")
    # per-device-row 1/count, [128, NQ]: recip[p, q] scales psum[q] row p
    recip = nc.dram_tensor("recip", [128, NQ], mybir.dt.float32,
                           kind="ExternalInput")
    # output c-major: [512 clusters, 256 batch] (fp16: |mean| < 1, the
    # 2^-11 quantization is far inside the error budget)
    out = nc.dram_tensor("out", [CLUSTERS_PER_CORE, B], mybir.dt.float16,
                         kind="ExternalOutput")

    xv, outv = x.ap(), out.ap()

    with tile.TileContext(nc) as tc:
        with (
            tc.tile_pool(name="xp", bufs=XBUFS) as xp,
            tc.tile_pool(name="ohp", bufs=1) as ohp,
            tc.tile_pool(name="ps", bufs=1, space="PSUM") as ps,
            tc.tile_pool(name="res", bufs=2) as resp,
        ):
            def body(_i=None):
                cidt = ohp.tile([128, nchunks], mybir.dt.bfloat16,
                                name="cidt", tag="cidt")
                nc.scalar.dma_start(cidt[:], cid.ap())
                iot = ohp.tile([128, G], mybir.dt.bfloat16,
                               name="iot", tag="iot")
                nc.scalar.dma_start(iot[:], iota.ap())
                rect = ohp.tile([128, NQ], mybir.dt.float32,
                                name="rect", tag="rect")
                nc.scalar.dma_start(rect[:], recip.ap())
                # expand to 0/1 one-hot weights (per group, so matmuls can
                # start as soon as the first slice is ready); fp8 chunks get
                # an fp8 one-hot, bf16 chunks a bf16 one
                oh8 = ohp.tile([128, GROUPS_PER_CORE * C8, G],
                               mybir.dt.float8e4, name="oh8", tag="oh8")
                oh16 = ohp.tile([128, GROUPS_PER_CORE * c16, G],
                                mybir.dt.bfloat16, name="oh16", tag="oh16")
                for g in range(GROUPS_PER_CORE):
                    nc.vector.tensor_tensor(
                        out=oh8[:, g * C8:(g + 1) * C8, :],
                        in0=cidt[:, g * cpg:g * cpg + C8].unsqueeze(2)
                            .broadcast_to([128, C8, G]),
                        in1=iot[:].unsqueeze(1).broadcast_to([128, C8, G]),
                        op=mybir.AluOpType.is_equal,
                    )
                    nc.vector.tensor_tensor(
                        out=oh16[:, g * c16:(g + 1) * c16, :],
                        in0=cidt[:, g * cpg + C8:(g + 1) * cpg].unsqueeze(2)
                            .broadcast_to([128, c16, G]),
                        in1=iot[:].unsqueeze(1).broadcast_to([128, c16, G]),
                        op=mybir.AluOpType.is_equal,
                    )
                psum = [
                    ps.tile([128, B], mybir.dt.float32,
                            name=f"psum{q}", tag=f"psum{q}")
                    for q in range(NQ)
                ]
                for f in range(nfetch):
                    xt = xp.tile([128, gbytes], mybir.dt.uint8, tag="xt")
                    # bulk fetches spread over three issue paths (two HWDGE
                    # rings + SWDGE) to hide per-DMA fixed cost; x is
                    # deep-prefetched so SWDGE's extra latency is harmless
                    eng = (nc.sync, nc.scalar, nc.gpsimd)[f % 3]
                    eng.dma_start(xt[:], xv[f][:, :])
                    g = f
                    q, gq = divmod(g, gpq)
                    po = gq * G        # partition offset within psum tile
                    for t in range(cpg):
                        if t < C8:
                            lhsT = oh8[:, g * C8 + t, :]
                            rhs = xt[:, t * B:(t + 1) * B].bitcast(
                                mybir.dt.float8e4)
                        else:
                            tb = t - C8
                            lhsT = oh16[:, g * c16 + tb, :]
                            rhs = xt[:, r8b + tb * B * 2:
                                     r8b + (tb + 1) * B * 2].bitcast(
                                mybir.dt.bfloat16)
                        nc.tensor.matmul(
                            out=psum[q][po:po + G, :],
                            lhsT=lhsT,
                            rhs=rhs,
                            start=(t == 0),
                            stop=(t == cpg - 1),
                            tile_position=(0, po),
                        )
                for q in range(NQ):
                    res = resp.tile([128, B], mybir.dt.float16,
                                    name=f"res{q}", tag="res")
                    nc.vector.tensor_tensor(
                        out=res[:],
                        in0=psum[q][:],
                        in1=rect[:, q:q + 1].broadcast_to([128, B]),
                        op=mybir.AluOpType.mult,
                    )
                    nc.scalar.dma_start(outv[q * 128:(q + 1) * 128, :],
                                        res[:])

            if loop == 1:
                body()
            else:
                with tc.For_i(0, loop, 1) as i:
                    body(i)

    nc.compile()
    _program_cache[key] = nc
    return nc


def _solve_bins(counts: np.ndarray):
    """Partition the 4096 clusters into 128 bins of exactly 32 clusters,
    equalizing bin row-sums (ideally all == 2048 -> zero padding). Returns
    (bin_of, slot_of) int arrays."""
    n_bins = N_CLUSTERS // G
    target = int(counts.sum()) // n_bins
    rng = np.random.default_rng(0)
    orderd = np.argsort(-counts)
    bins = [[] for _ in range(n_bins)]
    sums = np.zeros(n_bins, dtype=np.int64)
    nitems = np.zeros(n_bins, dtype=np.int64)
    for c in orderd:
        cand = np.where(nitems < G)[0]
        b = int(cand[np.argmin(sums[cand])])
        bins[b].append(int(c))
        sums[b] += counts[c]
        nitems[b] += 1
    for _ in range(300000):
        dev = sums - target
        over = np.where(dev > 0)[0]
        under = np.where(dev < 0)[0]
        if len(over) == 0 or len(under) == 0:
            break
        A = int(rng.choice(over))
        Bb = int(rng.choice(under))
        ca, cb = bins[A], bins[Bb]
        diff = counts[ca][:, None] - counts[cb][None, :]
        tot = np.abs(dev[A] - diff) + np.abs(dev[Bb] + diff)
        i, j = np.unravel_index(int(np.argmin(tot)), tot.shape)
        if tot[i, j] < abs(dev[A]) + abs(dev[Bb]):
            a, b2 = ca[i], cb[j]
            ca.remove(a), cb.remove(b2)
            ca.append(b2), cb.append(a)
            d = counts[a] - counts[b2]
            sums[A] -= d
            sums[Bb] += d
    bin_of = np.zeros(N_CLUSTERS, dtype=np.int64)
    slot_of = np.zeros(N_CLUSTERS, dtype=np.int64)
    for b, cl in enumerate(bins):
        bin_of[cl] = b
        slot_of[cl] = np.arange(len(cl))
    return bin_of, slot_of, int(sums.max())


def _prepare(output: np.ndarray, mapping: np.ndarray):
    """Host prep: returns (nc, in_maps, cpg, unperm)."""
    t0 = time.time()
    assert output.shape == (32, 8, 512, 512) and output.dtype == np.float32
    mapping = np.asarray(mapping).astype(np.int64).ravel()
    assert mapping.shape == (N,)

    data2d = output.reshape(B, N)
    counts = np.bincount(mapping, minlength=N_CLUSTERS).astype(np.int64)
    recip = (1.0 / np.maximum(counts, 1)).astype(np.float32)

    order = np.argsort(mapping, kind="stable")
    cum = np.zeros(N_CLUSTERS + 1, dtype=np.int64)
    np.cumsum(counts, out=cum[1:])

    n_groups = N_CLUSTERS // G
    # Bin-pack clusters into groups to minimize padding; fall back to
    # consecutive grouping if the packer leaves an oversized bin.
    bin_of, slot_of, maxsum = _solve_bins(counts)
    naive_max = int(np.add.reduceat(counts, np.arange(0, N_CLUSTERS, G)).max())
    if maxsum > naive_max:
        bin_of = np.arange(N_CLUSTERS) // G
        slot_of = np.arange(N_CLUSTERS) % G
        maxsum = naive_max
    cpg = max(1, int(np.ceil(maxsum / 128)))
    L = 128 * cpg

    # clusters in destination order (bin-major, slot order)
    dest_order = np.lexsort((slot_of, bin_of))
    glen = np.zeros(n_groups, dtype=np.int64)
    np.add.at(glen, bin_of, counts)
    rows_sorted = np.concatenate(
        [order[cum[c]:cum[c + 1]] for c in dest_order])
    gstart = np.zeros(n_groups + 1, dtype=np.int64)
    np.cumsum(glen, out=gstart[1:])

    # fp8/bf16 row assignment: a C8/cpg fraction of each cluster's rows,
    # striped evenly through the cluster, goes fp8 so quantization error
    # spreads across all clusters; per group the fp8 region is exactly
    # C8*128 rows (adjusted by promote/demote).
    run_len = counts[dest_order]
    run_start = np.concatenate([[0], np.cumsum(run_len)[:-1]])
    idx_within = np.arange(len(rows_sorted)) - np.repeat(run_start, run_len)
    sel8 = ((idx_within * C8) % cpg) < C8
    rows8_n = C8 * 128
    # Row-id table [n_groups, L]; -1 = padding. fp8 rows first, then bf16.
    pad_rows = np.full((n_groups, L), -1, dtype=np.int64)
    for g in range(n_groups):
        seg = rows_sorted[gstart[g]:gstart[g + 1]]
        s = sel8[gstart[g]:gstart[g + 1]].copy()
        c8n = int(s.sum())
        if c8n > rows8_n:                      # demote the excess
            on = np.flatnonzero(s)
            s[on[rows8_n:]] = False
        elif c8n < rows8_n:                    # promote unselected rows
            off = np.flatnonzero(~s)
            need = min(rows8_n - c8n, len(off))
            s[off[:need]] = True
        r8, r16 = seg[s], seg[~s]
        assert len(r8) <= rows8_n and len(r16) <= L - rows8_n
        pad_rows[g, :len(r8)] = r8
        pad_rows[g, rows8_n:rows8_n + len(r16)] = r16
    pad_rows = pad_rows.reshape(-1)        # [n_groups * L]
    vmask = pad_rows >= 0

    # Gather + quantize each region from fp32, then byte-pack per group:
    # per partition, [C8 chunks fp8 | cpg-C8 chunks bf16]
    dataT = np.ascontiguousarray(data2d.T)          # [N, B] fp32
    c16 = cpg - C8
    pr = pad_rows.reshape(n_groups, L)
    ids8 = pr[:, :rows8_n]                          # all valid
    m8 = ids8 >= 0
    a8 = np.zeros((n_groups, rows8_n, B), dtype=ml_dtypes.float8_e4m3)
    a8[m8] = dataT[ids8[m8]].astype(ml_dtypes.float8_e4m3)
    ids16 = pr[:, rows8_n:]
    m16 = ids16 >= 0
    a16 = np.zeros((n_groups, L - rows8_n, B), dtype=ml_dtypes.bfloat16)
    a16[m16] = dataT[ids16[m16]].astype(ml_dtypes.bfloat16)
    # Host emulation of the exact device result (quantized rows summed per
    # cluster, scaled, fp16-rounded) — used to detect transient device
    # corruption and retry. The device output is what gets returned.
    qflat = np.zeros((N, B), dtype=np.float32)
    qflat[ids8[m8]] = a8[m8].astype(np.float32)
    qflat[ids16[m16]] = a16[m16].astype(np.float32)
    a16 = a16.reshape(n_groups, c16, 128, B).transpose(0, 2, 1, 3)
    a16 = np.ascontiguousarray(a16).view(np.uint8).reshape(n_groups, 128, -1)
    a8 = a8.reshape(n_groups, C8, 128, B).transpose(0, 2, 1, 3)
    a8 = np.ascontiguousarray(a8).view(np.uint8).reshape(n_groups, 128, -1)
    x_all = np.concatenate([a8, a16], axis=2)       # [n_groups, 128, gbytes]
    # expected device output [4096, B] fp16, in device (dest_order) order
    qsorted = qflat[rows_sorted]
    sums_dev = np.add.reduceat(qsorted, run_start, axis=0)
    sums_dev[run_len == 0] = 0.0
    expect_dev = (sums_dev * recip[dest_order][:, None]).astype(np.float16)
    del qflat, qsorted

    # Compact one-hot: per-row within-group column id (padding rows get -1,
    # which matches no iota value -> all-zero one-hot row).
    cid_all = np.full(n_groups * L, -1.0, dtype=ml_dtypes.bfloat16)
    clus = mapping[pad_rows[vmask]]
    cid_all[vmask] = slot_of[clus].astype(np.float32)
    # where cluster c ended up in the concatenated [4096, B] device output
    unperm = bin_of * G + slot_of
    # per-device-row 1/count: device row (within core) = 32*g + slot,
    # packed [128, NQ] with row = q*128 + p
    recip_dev = recip[dest_order]          # [4096] in device order
    recip_pack = np.ascontiguousarray(
        recip_dev.reshape(NCORES, NQ, 128).transpose(0, 2, 1))
    # pack [rows] -> [core][p][chunk]
    nchunks = GROUPS_PER_CORE * cpg

    cid_all = np.ascontiguousarray(
        cid_all.reshape(NCORES, nchunks, 128).transpose(0, 2, 1))
    iota_np = np.broadcast_to(
        np.arange(G, dtype=np.float32).astype(ml_dtypes.bfloat16),
        (128, G)).copy()

    t1 = time.time()
    nc = _build_program(cpg)

    fpc = GROUPS_PER_CORE // FG    # fetches per core
    in_maps = []
    for k in range(NCORES):
        in_maps.append({
            "x": x_all[k * fpc:(k + 1) * fpc],
            "cid": cid_all[k],
            "iota": iota_np,
            "recip": recip_pack[k],
        })
    print(f"[kernel] host prep {t1 - t0:.2f}s  build+compile "
          f"{time.time() - t1:.2f}s  (cpg={cpg})", file=sys.stderr, flush=True)
    return nc, in_maps, cpg, unperm, expect_dev


def kernel(output: np.ndarray, mapping: np.ndarray) -> np.ndarray:
    nc, in_maps, _, unperm, expect_dev = _prepare(output, mapping)
    # Transient device/transport corruption has been observed (identical
    # program, wildly wrong values once in ~15 runs): verify the device
    # result against the host emulation of the same quantized computation
    # and retry on mismatch. The returned tensor is always device output.
    for attempt in range(4):
        t2 = time.time()
        res = run_bass_kernel_spmd(nc, in_maps, list(range(NCORES)))
        t3 = time.time()
        full = np.concatenate([np.asarray(res.results[k]["out"])
                               for k in range(NCORES)],
                              axis=0)               # [4096, 256] dev order
        dev_err = np.abs(full.astype(np.float32)
                         - expect_dev.astype(np.float32)).max()
        print(f"[kernel] run {t3 - t2:.2f}s  dev-vs-emul {dev_err:.2e}",
              file=sys.stderr, flush=True)
        if dev_err < 5e-3:
            break
        print(f"[kernel] device result corrupt (attempt {attempt}), "
              f"retrying", file=sys.stderr, flush=True)
    full = full.astype(np.float32)[unperm]          # -> cluster order
    out = np.ascontiguousarray(full.T).reshape(32, 8, N_CLUSTERS)
    return out


# revision 46
# speedup vs baseline: 1.1490x; 1.1490x over previous
"""Segment-mean (MeanToERA5) Trainium2 kernel.

Computes per-cluster means of a [32, 8, 512, 512] fp32 tensor over the
flattened 512x512 spatial axis, for 4096 clusters given by `mapping`
([262144] int), matching jax.ops.segment_sum(flat.T, mapping)/counts.

Strategy (8 NeuronCores, SPMD; the kernel is HBM-bandwidth bound):
  - Host: stable-argsort `mapping`; bin-pack the 4096 clusters into 128
    groups of G=32 with equal row sums (2048 -> zero padding); each core
    owns 512 clusters = 16 groups. Rows are laid out cluster-sorted and
    transposed as [256 batch] vectors, packed partition-major so every
    group is one fully contiguous HBM region fetched by one DMA.
  - Precision: the error gate is 2e-2. Every 4th row of each cluster is
    stored as fp8 e4m3, the rest as bf16 (measured end-to-end rel err
    1.44e-2, exactly reproduced on device since quantization happens on
    host and the device accumulates exactly in fp32 PSUM). This cuts HBM
    traffic to 14.7 MB/core: fp32 would be 33.5 MB, pure bf16 16.8 MB.
  - Device: build 0/1 one-hot weights on DVE from compact column-id
    vectors (fp8 and bf16 variants); per 128-row chunk one matmul:
    stationary = one-hot [128, 32], moving = data chunk [128, 256] viewed
    from the byte-packed tile via bitcast. PSUM accumulates [512 clusters,
    256 batch] c-major in 4 [128, 256] fp32 tiles; scale by per-cluster
    1/count on the psum->sbuf copy (Activation-ring DMAs for side inputs
    and outputs, x fetches alternate between both HWDGE rings), out fp16.
  - Host: assemble [4096, 256], unpermute, transpose (the unshard).
"""

import sys
import time

if "/opt/trn_rl_repo" not in sys.path:
    sys.path.insert(0, "/opt/trn_rl_repo")

import numpy as np
import ml_dtypes
import jax

# Persistent JAX compilation cache: the NEFF compile (~2 min) is reused
# across processes for identical programs.
try:
    if jax.config.jax_compilation_cache_dir is None:
        jax.config.update("jax_compilation_cache_dir", "/tmp/jax_neff_cache")
    jax.config.update("jax_persistent_cache_min_entry_size_bytes", -1)
    jax.config.update("jax_persistent_cache_min_compile_time_secs", 0.1)
except Exception:
    pass

import concourse.bacc as bacc
import concourse.tile as tile
from concourse import mybir
from concourse.bass_utils import run_bass_kernel_spmd

N_CLUSTERS = 4096
N = 512 * 512
B = 256
NCORES = 8
G = 32                      # clusters per group (= one-hot width)
GROUPS_PER_CORE = (N_CLUSTERS // NCORES) // G   # 16
CLUSTERS_PER_CORE = N_CLUSTERS // NCORES        # 512
NQ = CLUSTERS_PER_CORE // 128                   # psum tiles (4)
FG = 1                      # groups per x fetch
XBUFS = 13                  # x tile pool depth
C8 = 5                      # fp8 chunks per group (C8/16 of rows in e4m3;
                            # striped evenly per-cluster so every cluster is
                            # ~31% fp8 -> rel err ~1.47e-2, inside the 2e-2
                            # gate (C8=6 measured 1.79e-2: too thin)

_program_cache = {}
LAST_EXEC_NS = None


def _build_program(cpg: int, loop: int = 1):
    """Build the SPMD bass program for `cpg` 128-row chunks per group.

    loop > 1 repeats the whole pipeline on-device (for benchmarking: one
    dispatch, `loop` executions)."""
    key = (cpg, loop)
    if key in _program_cache:
        return _program_cache[key]

    nchunks = GROUPS_PER_CORE * cpg    # chunks per core
    gpq = 128 // G                     # groups per psum tile (4)

    nc = bacc.Bacc("TRN2", target_bir_lowering=False, debug=False,
                   num_devices=NCORES)
    # x packed per group as raw bytes: per partition, C8 fp8 chunks
    # (C8*B bytes) then (cpg-C8) bf16 chunks ((cpg-C8)*B*2 bytes);
    # host pre-permutes so every fetch is one contiguous region
    c16 = cpg - C8
    r8b = C8 * B                # fp8 region bytes per partition
    gbytes = r8b + c16 * B * 2  # total bytes per partition per group
    nfetch = GROUPS_PER_CORE // FG
    x = nc.dram_tensor("x", [nfetch, 128, gbytes],
                       mybir.dt.uint8, kind="ExternalInput")
    # per-row one-hot column id, packed [128, nchunks]
    cid = nc.dram_tensor("cid", [128, nchunks], mybir.dt.bfloat16,
                         kind="ExternalInput")
    iota = nc.dram_tensor("iota", [128, G], mybir.dt.bfloat16,
                          kind="ExternalInput")
    # per-device-row 1/count, [128, NQ]: recip[p, q] scales psum[q] row p
    recip = nc.dram_tensor("recip", [128, NQ], mybir.dt.float32,
                           kind="ExternalInput")
    # output c-major: [512 clusters, 256 batch] (fp16: |mean| < 1, the
    # 2^-11 quantization is far inside the error budget)
    out = nc.dram_tensor("out", [CLUSTERS_PER_CORE, B], mybir.dt.float16,
                         kind="ExternalOutput")

    xv, outv = x.ap(), out.ap()

    with tile.TileContext(nc) as tc:
        with (
            tc.tile_pool(name="xp", bufs=XBUFS) as xp,
            tc.tile_pool(name="ohp", bufs=1) as ohp,
            tc.tile_pool(name="ps", bufs=1, space="PSUM") as ps,
            tc.tile_pool(name="res", bufs=2) as resp,
        ):
            def body(_i=None):
                cidt = ohp.tile([128, nchunks], mybir.dt.bfloat16,
                                name="cidt", tag="cidt")
                nc.scalar.dma_start(cidt[:], cid.ap())
                iot = ohp.tile([128, G], mybir.dt.bfloat16,
                               name="iot", tag="iot")
                nc.scalar.dma_start(iot[:], iota.ap())
                rect = ohp.tile([128, NQ], mybir.dt.float32,
                                name="rect", tag="rect")
                nc.scalar.dma_start(rect[:], recip.ap())
                # expand to 0/1 one-hot weights (per group, so matmuls can
                # start as soon as the first slice is ready); fp8 chunks get
                # an fp8 one-hot, bf16 chunks a bf16 one
                oh8 = ohp.tile([128, GROUPS_PER_CORE * C8, G],
                               mybir.dt.float8e4, name="oh8", tag="oh8")
                oh16 = ohp.tile([128, GROUPS_PER_CORE * c16, G],
                                mybir.dt.bfloat16, name="oh16", tag="oh16")
                for g in range(GROUPS_PER_CORE):
                    nc.vector.tensor_tensor(
                        out=oh8[:, g * C8:(g + 1) * C8, :],
                        in0=cidt[:, g * cpg:g * cpg + C8].unsqueeze(2)
                            .broadcast_to([128, C8, G]),
                        in1=iot[:].unsqueeze(1).broadcast_to([128, C8, G]),
                        op=mybir.AluOpType.is_equal,
                    )
                    nc.vector.tensor_tensor(
                        out=oh16[:, g * c16:(g + 1) * c16, :],
                        in0=cidt[:, g * cpg + C8:(g + 1) * cpg].unsqueeze(2)
                            .broadcast_to([128, c16, G]),
                        in1=iot[:].unsqueeze(1).broadcast_to([128, c16, G]),
                        op=mybir.AluOpType.is_equal,
                    )
                psum = [
                    ps.tile([128, B], mybir.dt.float32,
                            name=f"psum{q}", tag=f"psum{q}")
                    for q in range(NQ)
                ]
                for f in range(nfetch):
                    xt = xp.tile([128, gbytes], mybir.dt.uint8, tag="xt")
                    eng = nc.sync if f % 2 == 0 else nc.scalar
                    eng.dma_start(xt[:], xv[f][:, :])
                    g = f
                    q, gq = divmod(g, gpq)
                    po = gq * G        # partition offset within psum tile
                    for t in range(cpg):
                        if t < C8:
                            lhsT = oh8[:, g * C8 + t, :]
                            rhs = xt[:, t * B:(t + 1) * B].bitcast(
                                mybir.dt.float8e4)
                        else:
                            tb = t - C8
                            lhsT = oh16[:, g * c16 + tb, :]
                            rhs = xt[:, r8b + tb * B * 2:
                                     r8b + (tb + 1) * B * 2].bitcast(
                                mybir.dt.bfloat16)
                        nc.tensor.matmul(
                            out=psum[q][po:po + G, :],
                            lhsT=lhsT,
                            rhs=rhs,
                            start=(t == 0),
                            stop=(t == cpg - 1),
                            tile_position=(0, po),
                        )
                for q in range(NQ):
                    res = resp.tile([128, B], mybir.dt.float16,
                                    name=f"res{q}", tag="res")
                    nc.vector.tensor_tensor(
                        out=res[:],
                        in0=psum[q][:],
                        in1=rect[:, q:q + 1].broadcast_to([128, B]),
                        op=mybir.AluOpType.mult,
                    )
                    nc.scalar.dma_start(outv[q * 128:(q + 1) * 128, :],
                                        res[:])

            if loop == 1:
                body()
            else:
                with tc.For_i(0, loop, 1) as i:
                    body(i)

    nc.compile()
    _program_cache[key] = nc
    return nc


def _solve_bins(counts: np.ndarray):
    """Partition the 4096 clusters into 128 bins of exactly 32 clusters,
    equalizing bin row-sums (ideally all == 2048 -> zero padding). Returns
    (bin_of, slot_of) int arrays."""
    n_bins = N_CLUSTERS // G
    target = int(counts.sum()) // n_bins
    rng = np.random.default_rng(0)
    orderd = np.argsort(-counts)
    bins = [[] for _ in range(n_bins)]
    sums = np.zeros(n_bins, dtype=np.int64)
    nitems = np.zeros(n_bins, dtype=np.int64)
    for c in orderd:
        cand = np.where(nitems < G)[0]
        b = int(cand[np.argmin(sums[cand])])
        bins[b].append(int(c))
        sums[b] += counts[c]
        nitems[b] += 1
    for _ in range(300000):
        dev = sums - target
        over = np.where(dev > 0)[0]
        under = np.where(dev < 0)[0]
        if len(over) == 0 or len(under) == 0:
            break
        A = int(rng.choice(over))
        Bb = int(rng.choice(under))
        ca, cb = bins[A], bins[Bb]
        diff = counts[ca][:, None] - counts[cb][None, :]
        tot = np.abs(dev[A] - diff) + np.abs(dev[Bb] + diff)
        i, j = np.unravel_index(int(np.argmin(tot)), tot.shape)
        if tot[i, j] < abs(dev[A]) + abs(dev[Bb]):
            a, b2 = ca[i], cb[j]
            ca.remove(a), cb.remove(b2)
            ca.append(b2), cb.append(a)
            d = counts[a] - counts[b2]
            sums[A] -= d
            sums[Bb] += d
    bin_of = np.zeros(N_CLUSTERS, dtype=np.int64)
    slot_of = np.zeros(N_CLUSTERS, dtype=np.int64)
    for b, cl in enumerate(bins):
        bin_of[cl] = b
        slot_of[cl] = np.arange(len(cl))
    return bin_of, slot_of, int(sums.max())


def _prepare(output: np.ndarray, mapping: np.ndarray):
    """Host prep: returns (nc, in_maps, cpg, unperm)."""
    t0 = time.time()
    assert output.shape == (32, 8, 512, 512) and output.dtype == np.float32
    mapping = np.asarray(mapping).astype(np.int64).ravel()
    assert mapping.shape == (N,)

    data2d = output.reshape(B, N)
    counts = np.bincount(mapping, minlength=N_CLUSTERS).astype(np.int64)
    recip = (1.0 / np.maximum(counts, 1)).astype(np.float32)

    order = np.argsort(mapping, kind="stable")
    cum = np.zeros(N_CLUSTERS + 1, dtype=np.int64)
    np.cumsum(counts, out=cum[1:])

    n_groups = N_CLUSTERS // G
    # Bin-pack clusters into groups to minimize padding; fall back to
    # consecutive grouping if the packer leaves an oversized bin.
    bin_of, slot_of, maxsum = _solve_bins(counts)
    naive_max = int(np.add.reduceat(counts, np.arange(0, N_CLUSTERS, G)).max())
    if maxsum > naive_max:
        bin_of = np.arange(N_CLUSTERS) // G
        slot_of = np.arange(N_CLUSTERS) % G
        maxsum = naive_max
    cpg = max(1, int(np.ceil(maxsum / 128)))
    L = 128 * cpg

    # clusters in destination order (bin-major, slot order)
    dest_order = np.lexsort((slot_of, bin_of))
    glen = np.zeros(n_groups, dtype=np.int64)
    np.add.at(glen, bin_of, counts)
    rows_sorted = np.concatenate(
        [order[cum[c]:cum[c + 1]] for c in dest_order])
    gstart = np.zeros(n_groups + 1, dtype=np.int64)
    np.cumsum(glen, out=gstart[1:])

    # fp8/bf16 row assignment: a C8/cpg fraction of each cluster's rows,
    # striped evenly through the cluster, goes fp8 so quantization error
    # spreads across all clusters; per group the fp8 region is exactly
    # C8*128 rows (adjusted by promote/demote).
    run_len = counts[dest_order]
    run_start = np.concatenate([[0], np.cumsum(run_len)[:-1]])
    idx_within = np.arange(len(rows_sorted)) - np.repeat(run_start, run_len)
    sel8 = ((idx_within * C8) % cpg) < C8
    rows8_n = C8 * 128
    # Row-id table [n_groups, L]; -1 = padding. fp8 rows first, then bf16.
    pad_rows = np.full((n_groups, L), -1, dtype=np.int64)
    for g in range(n_groups):
        seg = rows_sorted[gstart[g]:gstart[g + 1]]
        s = sel8[gstart[g]:gstart[g + 1]].copy()
        c8n = int(s.sum())
        if c8n > rows8_n:                      # demote the excess
            on = np.flatnonzero(s)
            s[on[rows8_n:]] = False
        elif c8n < rows8_n:                    # promote unselected rows
            off = np.flatnonzero(~s)
            need = min(rows8_n - c8n, len(off))
            s[off[:need]] = True
        r8, r16 = seg[s], seg[~s]
        assert len(r8) <= rows8_n and len(r16) <= L - rows8_n
        pad_rows[g, :len(r8)] = r8
        pad_rows[g, rows8_n:rows8_n + len(r16)] = r16
    pad_rows = pad_rows.reshape(-1)        # [n_groups * L]
    vmask = pad_rows >= 0

    # Gather + quantize each region from fp32, then byte-pack per group:
    # per partition, [C8 chunks fp8 | cpg-C8 chunks bf16]
    dataT = np.ascontiguousarray(data2d.T)          # [N, B] fp32
    c16 = cpg - C8
    pr = pad_rows.reshape(n_groups, L)
    ids8 = pr[:, :rows8_n]                          # all valid
    m8 = ids8 >= 0
    a8 = np.zeros((n_groups, rows8_n, B), dtype=ml_dtypes.float8_e4m3)
    a8[m8] = dataT[ids8[m8]].astype(ml_dtypes.float8_e4m3)
    ids16 = pr[:, rows8_n:]
    m16 = ids16 >= 0
    a16 = np.zeros((n_groups, L - rows8_n, B), dtype=ml_dtypes.bfloat16)
    a16[m16] = dataT[ids16[m16]].astype(ml_dtypes.bfloat16)
    # Host emulation of the exact device result (quantized rows summed per
    # cluster, scaled, fp16-rounded) — used to detect transient device
    # corruption and retry. The device output is what gets returned.
    qflat = np.zeros((N, B), dtype=np.float32)
    qflat[ids8[m8]] = a8[m8].astype(np.float32)
    qflat[ids16[m16]] = a16[m16].astype(np.float32)
    a16 = a16.reshape(n_groups, c16, 128, B).transpose(0, 2, 1, 3)
    a16 = np.ascontiguousarray(a16).view(np.uint8).reshape(n_groups, 128, -1)
    a8 = a8.reshape(n_groups, C8, 128, B).transpose(0, 2, 1, 3)
    a8 = np.ascontiguousarray(a8).view(np.uint8).reshape(n_groups, 128, -1)
    x_all = np.concatenate([a8, a16], axis=2)       # [n_groups, 128, gbytes]
    # expected device output [4096, B] fp16, in device (dest_order) order
    qsorted = qflat[rows_sorted]
    sums_dev = np.add.reduceat(qsorted, run_start, axis=0)
    sums_dev[run_len == 0] = 0.0
    expect_dev = (sums_dev * recip[dest_order][:, None]).astype(np.float16)
    del qflat, qsorted

    # Compact one-hot: per-row within-group column id (padding rows get -1,
    # which matches no iota value -> all-zero one-hot row).
    cid_all = np.full(n_groups * L, -1.0, dtype=ml_dtypes.bfloat16)
    clus = mapping[pad_rows[vmask]]
    cid_all[vmask] = slot_of[clus].astype(np.float32)
    # where cluster c ended up in the concatenated [4096, B] device output
    unperm = bin_of * G + slot_of
    # per-device-row 1/count: device row (within core) = 32*g + slot,
    # packed [128, NQ] with row = q*128 + p
    recip_dev = recip[dest_order]          # [4096] in device order
    recip_pack = np.ascontiguousarray(
        recip_dev.reshape(NCORES, NQ, 128).transpose(0, 2, 1))
    # pack [rows] -> [core][p][chunk]
    nchunks = GROUPS_PER_CORE * cpg

    cid_all = np.ascontiguousarray(
        cid_all.reshape(NCORES, nchunks, 128).transpose(0, 2, 1))
    iota_np = np.broadcast_to(
        np.arange(G, dtype=np.float32).astype(ml_dtypes.bfloat16),
        (128, G)).copy()

    t1 = time.time()
    nc = _build_program(cpg)

    fpc = GROUPS_PER_CORE // FG    # fetches per core
    in_maps = []
    for k in range(NCORES):
        in_maps.append({
            "x": x_all[k * fpc:(k + 1) * fpc],
            "cid": cid_all[k],
            "iota": iota_np,
            "recip": recip_pack[k],
        })
    print(f"[kernel] host prep {t1 - t0:.2f}s  build+compile "
          f"{time.time() - t1:.2f}s  (cpg={cpg})", file=sys.stderr, flush=True)
    return nc, in_maps, cpg, unperm, expect_dev


def kernel(output: np.ndarray, mapping: np.ndarray) -> np.ndarray:
    nc, in_maps, _, unperm, expect_dev = _prepare(output, mapping)
    # Transient device/transport corruption has been observed (identical
    # program, wildly wrong values once in ~15 runs): verify the device
    # result against the host emulation of the same quantized computation
    # and retry on mismatch. The returned tensor is always device output.
    for attempt in range(4):
        t2 = time.time()
        res = run_bass_kernel_spmd(nc, in_maps, list(range(NCORES)))
        t3 = time.time()
        full = np.concatenate([np.asarray(res.results[k]["out"])
                               for k in range(NCORES)],
                              axis=0)               # [4096, 256] dev order
        dev_err = np.abs(full.astype(np.float32)
                         - expect_dev.astype(np.float32)).max()
        print(f"[kernel] run {t3 - t2:.2f}s  dev-vs-emul {dev_err:.2e}",
              file=sys.stderr, flush=True)
        if dev_err < 5e-3:
            break
        print(f"[kernel] device result corrupt (attempt {attempt}), "
              f"retrying", file=sys.stderr, flush=True)
    full = full.astype(np.float32)[unperm]          # -> cluster order
    out = np.ascontiguousarray(full.T).reshape(32, 8, N_CLUSTERS)
    return out
